# revision 58
# baseline (speedup 1.0000x reference)
"""BERT-base forward on 8 Trainium2 NeuronCores, data-parallel over batch.

Each core runs the full 12-layer model on one batch element (512 tokens).
v2: all matmul operands in fp16 (1 cyc/row on PE, FWL weight loads, half the
weight DMA of f32r; rel-err ~6e-4 vs fp32 reference). Residual stream stays
fp32 token-major; hidden-major operands (xT/yT/QT/KT/attnT/h1T) are fp16.

Key structure per layer / core (SBUF tiles are [128 partitions, ...]):
  x token-major f32 [128, TT, H] -> PE-transpose (f32r) -> xT f16 [128, HC, S]
  QT/KT f16 via 6x6 128-blocked matmuls, evicted from 2-bank PSUM pairs.
  V token-major f16 [128, TT, H].
  Attention per head pair c: scores for heads (2c, 2c+1) are row-packed
  (tile_position rows 0:64 / 64:128) into one 2-bank PSUM tile per k-chunk;
  ONE Exp per [128, 2*S] tile (halves ACT op overhead); denominators and
  O^T=V'expS are column-packed pairs (out partitions 0:64 / 64:128, separate
  banks) so the two heads' matmuls run concurrently in the PE array.
  Wo/FFN-down accumulate (n=0,1) halves into one 2-bank PSUM tile; eviction is
  a single scalar_tensor_tensor that also adds the residual AND produces the
  LN row-sum via accum_out. LN variance via ACT Square+accum; rstd via
  bit-trick + 2 Newton steps on DVE (no Sqrt -> no ACT table switch); the
  normalize runs on GpSimd to unload DVE. Exp/Gelu are the only table sets.

Work that is provably a no-op for the given inputs (zero biases, unit gammas,
zero betas, all-ones mask) is skipped at build time; general paths stay
available and are selected per-input on the host.
"""
import os
import numpy as np
import ml_dtypes
from contextlib import ExitStack

import concourse.bass as bass
import concourse.tile as tile
from concourse import bacc, mybir
from concourse import bass_utils

f32 = mybir.dt.float32
f32r = mybir.dt.float32r
f16 = mybir.dt.float16
i32 = mybir.dt.int32
AF = mybir.ActivationFunctionType
OP = mybir.AluOpType
AX = mybir.AxisListType

V, H, L, NH, I, P, B, S = 30000, 768, 12, 12, 3072, 512, 8, 512
D = H // NH          # 64
HC = H // 128        # 6 hidden chunks
FC = I // 128        # 24 ffn chunks
TT = S // 128        # 4 token tiles
LN_EPS = 1e-3
MAGIC = 0x5F3759DF

LAST_EXEC_TIME_NS = None
LAST_RESULT = None


def _act_hoist(nc, pools, func):
    """Tiny ACT op that forces the table set for `func` to load here (off the
    critical path) instead of right before the first real use."""
    one = pools["act_one"]
    j = pools["vec"].tile([128, 1], f32, tag="v", name="act_pre")
    nc.scalar.activation(j[:], one[:], func)


def _ln_pair(nc, pools, z, s4, tts, g_bc, b_bc):
    """LayerNorm (in place, over hidden) for token tiles `tts` of z.

    z [128, TT, H] f16; s4 [128, 4] holds per-tile row sums (cols = tt) already
    accumulated by the evictions. Processing tile-pairs right after their
    evictions keeps this chain off the critical path of the following phase.
    rstd comes from ACT Sqrt + DVE reciprocal; the sqrt table set is preloaded
    off the critical path via _act_hoist.
    """
    vec, scratch = pools["vec"], pools["scratch"]
    w = len(tts)
    t0 = tts[0]
    sp = s4[:, t0:t0 + w]
    ssq = vec.tile([128, w], f32, tag="v", name="ln_ssq")
    for i, tt in enumerate(tts):
        sq = scratch.tile([128, H], f32, tag="sc", name="ln_sq")
        nc.scalar.activation(sq[:], z[:, tt, :], AF.Square,
                             accum_out=ssq[:, i:i + 1])
    b2 = vec.tile([128, w], f32, tag="v", name="ln_b2")
    nc.vector.scalar_tensor_tensor(out=b2[:], in0=sp,
                                   scalar=float(-1.0 / (H * H)), in1=sp,
                                   op0=OP.mult, op1=OP.mult)
    nc.vector.tensor_scalar(out=b2[:], in0=b2[:], scalar1=float(LN_EPS),
                            scalar2=None, op0=OP.add)
    sd = vec.tile([128, w], f32, tag="v", name="ln_sd")
    for i in range(w):
        nc.scalar.activation(sd[:, i:i + 1], ssq[:, i:i + 1], AF.Sqrt,
                             bias=b2[:, i:i + 1], scale=float(1.0 / H))
    r = vec.tile([128, w], f32, tag="v", name="ln_r")
    nc.vector.reciprocal(r[:], sd[:])
    mr = vec.tile([128, w], f32, tag="v", name="ln_mr")
    nc.vector.scalar_tensor_tensor(out=mr[:], in0=sp,
                                   scalar=float(-1.0 / H), in1=r[:],
                                   op0=OP.mult, op1=OP.mult)
    for i, tt in enumerate(tts):
        eng = nc.vector if (tt % 2 == 0) else nc.gpsimd
        eng.tensor_scalar(out=z[:, tt, :], in0=z[:, tt, :],
                          scalar1=r[:, i:i + 1], scalar2=mr[:, i:i + 1],
                          op0=OP.mult, op1=OP.add)
        if g_bc is not None:
            nc.vector.tensor_tensor(out=z[:, tt, :], in0=z[:, tt, :],
                                    in1=g_bc[:], op=OP.mult)
        if b_bc is not None:
            nc.vector.tensor_tensor(out=z[:, tt, :], in0=z[:, tt, :],
                                    in1=b_bc[:], op=OP.add)


def _ln_apply(nc, pools, z, s4, g_bc, b_bc):
    _ln_pair(nc, pools, z, s4, (0, 1), g_bc, b_bc)
    _ln_pair(nc, pools, z, s4, (2, 3), g_bc, b_bc)


def _ln_bcast(nc, pools, g_row, b_row, affine):
    if not affine:
        return None, None
    gb = pools["gb"]
    g_bc = gb.tile([128, H], f32, tag="gb", name="g_bc")
    nc.sync.dma_start(g_bc[:], g_row[None, :].partition_broadcast(128))
    b_bc = gb.tile([128, H], f32, tag="gb", name="b_bc")
    nc.sync.dma_start(b_bc[:], b_row[None, :].partition_broadcast(128))
    return g_bc, b_bc


def _transpose_half(nc, pools, src, dst, ident, tts):
    """Transpose token tiles `tts` (a (0,1) or (2,3) pair) of token-major src
    [128, TT, H] f16 into the matching column half of hidden-major dst
    [128, HC, S] f16. Split by halves so the (0,1) half runs as soon as its
    LayerNorm pair lands, under the tail of the producing phase."""
    psm1 = pools["psm1"]
    t0 = tts[0]
    for c in range(HC):
        tp = psm1.tile([128, 256], f32, tag="m1", name="tp")
        for i, tt in enumerate(tts):
            # out = src_block.T via plain matmul with identity as the moving
            # operand: out[m, n] = sum_p src[p, m] * I[p, n] = src[n, m].
            nc.tensor.matmul(tp[:, i * 128:(i + 1) * 128],
                             lhsT=src[:, tt, c * 128:c * 128 + 128],
                             rhs=ident[:], start=True, stop=True)
        nc.vector.tensor_copy(dst[:, c, t0 * 128:t0 * 128 + 256], tp[:])


def _transpose_into(nc, pools, src, dst, ident):
    _transpose_half(nc, pools, src, dst, ident, (0, 1))
    _transpose_half(nc, pools, src, dst, ident, (2, 3))


def _dbg_dump(nc, dbg_d, src_ap, n):
    """DMA an SBUF view with free size n to the raw debug output."""
    nc.sync.dma_start(dbg_d[:, 0:n], src_ap)


def build(n_layers=L, flags=None):
    fl = flags or {}
    qk_bias = fl.get("qk_bias", True)
    v_bias = fl.get("v_bias", True)
    o_bias = fl.get("o_bias", True)
    i_bias = fl.get("i_bias", True)
    d_bias = fl.get("d_bias", True)
    ln1_aff = fl.get("ln1_aff", True)
    ln2_aff = fl.get("ln2_aff", True)
    emb_aff = fl.get("emb_aff", True)
    use_mask = fl.get("use_mask", True)
    use_type = fl.get("use_type", True)

    nc = bacc.Bacc("TRN2", target_bir_lowering=False, debug=False, num_devices=8)

    dt_in = lambda n, s, d: nc.dram_tensor(n, s, d, kind="ExternalInput").ap()
    ids_d = dt_in("ids", [S], i32)
    tti_d = dt_in("tti", [S], i32)
    mb_d = dt_in("mb", [S], f32)
    tok_d = dt_in("tok_emb", [V, H], f32)
    pos_d = dt_in("pos_emb", [S, H], f32)
    typ_d = dt_in("type_emb", [2, H], f32)
    eg_d = dt_in("emb_g", [H], f32)
    eb_d = dt_in("emb_b", [H], f32)
    wq_d = dt_in("Wq16", [L, HC, 128, HC, 128], f16)
    wk_d = dt_in("Wk16", [L, HC, 128, HC, 128], f16)
    wv_d = dt_in("Wv16", [L, 128, HC, 768], f16)
    wo_d = dt_in("Wo16", [L, 128, HC, 768], f16)
    wi_d = dt_in("Wi16", [L, FC // 2, 128, 2, HC, 128], f16)
    wd_d = dt_in("Wd16", [L, FC // 4, 128, 4, H], f16)
    bq_d = dt_in("bq", [L, H], f32)
    bk_d = dt_in("bk", [L, H], f32)
    bv_d = dt_in("bv", [L, H], f32)
    bo_d = dt_in("bo", [L, H], f16)
    bi_d = dt_in("bi", [L, I], f32)
    bd_d = dt_in("bd", [L, H], f16)
    g1_d = dt_in("ln1_g", [L, H], f32)
    b1_d = dt_in("ln1_b", [L, H], f32)
    g2_d = dt_in("ln2_g", [L, H], f32)
    b2_d = dt_in("ln2_b", [L, H], f32)
    ones_d = dt_in("ones16", [128, 128], f16)
    ident_d = dt_in("ident", [128, 128], f16)
    out_d = nc.dram_tensor("out", [S, H], f32, kind="ExternalOutput").ap()
    dbg = os.environ.get("BERT_DBG")
    dbg_d = None
    if dbg:
        dbg_dt = f32 if dbg in ("dnav",) else f16
        dbg_d = nc.dram_tensor("dbg", [128, FC * S], dbg_dt,
                               kind="ExternalOutput").ap()

    with tile.TileContext(nc) as tc, ExitStack() as ctx:
        acts16 = ctx.enter_context(tc.tile_pool(name="acts16", bufs=12))
        h1p = ctx.enter_context(tc.tile_pool(name="h1p", bufs=1))
        wqk = ctx.enter_context(tc.tile_pool(name="wqk", bufs=6))
        wvo = ctx.enter_context(tc.tile_pool(name="wvo", bufs=2))
        wip = ctx.enter_context(tc.tile_pool(name="wip", bufs=3))
        wdp = ctx.enter_context(tc.tile_pool(name="wdp", bufs=3))
        esp = ctx.enter_context(tc.tile_pool(name="esp", bufs=14))
        bcp = ctx.enter_context(tc.tile_pool(name="bcp", bufs=4))
        gb = ctx.enter_context(tc.tile_pool(name="gb", bufs=2))
        scratch = ctx.enter_context(tc.tile_pool(name="scratch", bufs=2))
        vec = ctx.enter_context(tc.tile_pool(name="vec", bufs=28))
        brow_p = ctx.enter_context(tc.tile_pool(name="brow_p", bufs=2))
        const = ctx.enter_context(tc.tile_pool(name="const", bufs=1))
        psm1 = ctx.enter_context(tc.tile_pool(name="psm1", bufs=2, space="PSUM"))
        psm2 = ctx.enter_context(tc.tile_pool(name="psm2", bufs=3, space="PSUM"))

        pools = dict(gb=gb, vec=vec, scratch=scratch, psm1=psm1, psm2=psm2)

        # constants
        ones_sb = const.tile([128, 128], f16, tag="ones", name="ones_sb")
        nc.sync.dma_start(ones_sb[:], ones_d[:])
        ident = const.tile([128, 128], f16, tag="ident", name="ident")
        nc.sync.dma_start(ident[:], ident_d[:])
        act_one = const.tile([128, 1], f32, tag="aone", name="act_one")
        nc.vector.memset(act_one[:], 1.0)
        pools["act_one"] = act_one
        ids_sb = const.tile([128, TT], i32, tag="ids", name="ids_sb")
        nc.sync.dma_start(ids_sb[:], ids_d.rearrange("(t p) -> p t", p=128))
        if use_type:
            tti_sb = const.tile([128, TT], i32, tag="tti", name="tti_sb")
            nc.sync.dma_start(tti_sb[:], tti_d.rearrange("(t p) -> p t", p=128))
        if use_mask:
            mb_sb = const.tile([128, TT], f32, tag="mb", name="mb_sb")
            nc.sync.dma_start(mb_sb[:], mb_d.rearrange("(t p) -> p t", p=128))

        # ---- embedding ----
        x = acts16.tile([128, TT, H], f16, tag="act16", name="x_emb")
        eg_bc, eb_bc = _ln_bcast(nc, pools, eg_d, eb_d, emb_aff)
        s4e = vec.tile([128, 4], f32, tag="v", name="s4_emb")
        for tt in range(TT):
            xg = scratch.tile([128, H], f32, tag="sc", name="emb_gather")
            nc.gpsimd.indirect_dma_start(
                out=xg[:], out_offset=None, in_=tok_d[:],
                in_offset=bass.IndirectOffsetOnAxis(ap=ids_sb[:, tt:tt + 1], axis=0))
            if use_type:
                tmp_t = gb.tile([128, H], f32, tag="gb", name="emb_tmp")
                nc.gpsimd.indirect_dma_start(
                    out=tmp_t[:], out_offset=None, in_=typ_d[:],
                    in_offset=bass.IndirectOffsetOnAxis(ap=tti_sb[:, tt:tt + 1], axis=0))
                nc.vector.tensor_tensor(out=xg[:], in0=xg[:], in1=tmp_t[:],
                                        op=OP.add)
            tmp_p = gb.tile([128, H], f32, tag="gb", name="emb_pos")
            nc.sync.dma_start(tmp_p[:], pos_d[tt * 128:(tt + 1) * 128, :])
            nc.vector.scalar_tensor_tensor(out=x[:, tt, :], in0=tmp_p[:],
                                           scalar=1.0, in1=xg[:],
                                           op0=OP.mult, op1=OP.add,
                                           accum_out=s4e[:, tt:tt + 1])
        _ln_apply(nc, pools, x, s4e, eg_bc, eb_bc)
        xT = acts16.tile([128, HC, S], f16, tag="act16", name="xT_emb")
        _transpose_into(nc, pools, x, xT, ident)

        # ---- layers ----
        for l in range(n_layers):
            # V token-major f16 [128, TT, H] (needed by the per-chunk attention)
            Vt = acts16.tile([128, TT, H], f16, tag="act16", name=f"V_{l}")
            wvblk = wvo.tile([128, HC, 768], f16, tag="wvo", name="wv_blk")
            nc.sync.dma_start(wvblk[:], wv_d[l])
            for tt in range(TT):
                pv = psm2.tile([128, 2, S], f32, tag="m2", name="pv")
                for n in range(2):
                    for ic in range(HC):
                        nc.tensor.matmul(
                            pv[:, n, 0:384],
                            lhsT=xT[:, ic, tt * 128:tt * 128 + 128],
                            rhs=wvblk[:, ic, n * 384:(n + 1) * 384],
                            start=(ic == 0), stop=(ic == HC - 1))
                nc.vector.tensor_copy(Vt[:, tt, :], pv[:, :, 0:384])

            # Q^T/K^T production interleaved with per-head-pair attention:
            # after chunk pair jj of both Q and K is evicted, attention for
            # head pairs c = 2jj, 2jj+1 runs (its exps keep ACT busy while the
            # PE streams the next jj's projections).
            QT = acts16.tile([128, HC, S], f16, tag="act16", name=f"QT_{l}")
            KT = acts16.tile([128, HC, S], f16, tag="act16", name=f"KT_{l}")
            attnT = acts16.tile([128, HC, S], f16, tag="act16", name=f"attnT_{l}")
            for jj in range(HC // 2):
                for dst, w_d, b_d in ((QT, wq_d, bq_d), (KT, wk_d, bk_d)):
                    acc = psm2.tile([128, 2, S], f32, tag="m2", name="qk_acc")
                    for j2 in range(2):
                        j = 2 * jj + j2
                        wblk = wqk.tile([128, HC, 128], f16, tag="wqk",
                                        name="wqk_blk")
                        nc.sync.dma_start(wblk[:], w_d[l, j])
                        for ic in range(HC):
                            nc.tensor.matmul(acc[:, j2, :], lhsT=wblk[:, ic, :],
                                             rhs=xT[:, ic, :],
                                             start=(ic == 0), stop=(ic == HC - 1))
                    if qk_bias:
                        for j2 in range(2):
                            j = 2 * jj + j2
                            b_sl = vec.tile([128, 1], f32, tag="v", name="bqk_sl")
                            nc.sync.dma_start(
                                b_sl[:], b_d[l, j * 128:(j + 1) * 128][:, None])
                            nc.scalar.activation(dst[:, j, :], acc[:, j2, :],
                                                 AF.Identity, bias=b_sl[:])
                    else:
                        nc.vector.tensor_copy(dst[:, 2 * jj:2 * jj + 2, :], acc[:])
                for c in (2 * jj, 2 * jj + 1):
                    # dn/av accumulate column-packed (head A rows 0:64, head B
                    # rows 64:128, concurrent via col groups). The PE stream is
                    # software-pipelined at k-chunk granularity: scores run two
                    # chunks ahead of the dn/av consumers so the PE never
                    # head-of-line blocks on an Exp result.
                    dn = psm1.tile([128, S], f32, tag="m1", name="dn")
                    av = psm1.tile([128, S], f32, tag="m1", name="av")
                    es = [None] * TT

                    def emit_scores(kc, c=c, es=es):
                        sp = psm2.tile([128, 2, S], f32, tag="m2", name="sp")
                        for hh in range(2):
                            r0 = 64 * hh
                            nc.tensor.matmul(
                                sp[:, hh, :],
                                lhsT=KT[r0:r0 + 64, c, kc * 128:kc * 128 + 128],
                                rhs=QT[r0:r0 + 64, c, :],
                                start=True, stop=True)
                        e = esp.tile([128, 2, S], f16, tag="es", name=f"e{kc}")
                        if use_mask:
                            mbias = mb_sb[:, kc:kc + 1]
                            for hh in range(2):
                                nc.scalar.activation(e[:, hh, :], sp[:, hh, :],
                                                     AF.Exp, bias=mbias,
                                                     scale=0.125)
                        else:
                            nc.scalar.activation(e[:], sp[:], AF.Exp, scale=0.125)
                        es[kc] = e

                    def emit_dnav(kc, c=c, es=es, dn=dn, av=av):
                        for hh in range(2):
                            nc.tensor.matmul(dn[64 * hh:64 * hh + 64, :],
                                             lhsT=ones_sb[:, 0:64],
                                             rhs=es[kc][:, hh, :],
                                             start=(kc == 0), stop=(kc == TT - 1))
                        for hh in range(2):
                            nc.tensor.matmul(
                                av[64 * hh:64 * hh + 64, :],
                                lhsT=Vt[:, kc,
                                        (2 * c + hh) * D:(2 * c + hh + 1) * D],
                                rhs=es[kc][:, hh, :],
                                start=(kc == 0), stop=(kc == TT - 1))

                    emit_scores(0)
                    emit_scores(1)
                    emit_dnav(0)
                    emit_scores(2)
                    emit_dnav(1)
                    emit_scores(3)
                    emit_dnav(2)
                    emit_dnav(3)
                    bct = bcp.tile([128, S], f32, tag="bc", name="bct")
                    nc.vector.reciprocal_approx_fast(out=bct[:], in_=dn[:])
                    nc.vector.tensor_tensor(out=attnT[:, c, :], in0=av[:],
                                            in1=bct[:], op=OP.mult)
                    if v_bias:
                        bv_sl = vec.tile([128, 1], f32, tag="v", name="bv_sl")
                        nc.sync.dma_start(
                            bv_sl[:], bv_d[l, 2 * c * D:(2 * c + 2) * D][:, None])
                        nc.vector.tensor_scalar(
                            out=attnT[:, c, :], in0=attnT[:, c, :].bitcast(f16),
                            scalar1=bv_sl[:], scalar2=None, op0=OP.add)

            if dbg and l == 0 and dbg in ("xt", "qt", "kt"):
                src = {"xt": xT, "qt": QT, "kt": KT}[dbg]
                _dbg_dump(nc, dbg_d, src[:], HC * S)
            if dbg and l == 0 and dbg in ("vt", "attnt"):
                src = {"vt": Vt, "attnt": attnT}[dbg]
                _dbg_dump(nc, dbg_d, src[:], TT * H)

            _act_hoist(nc, pools, AF.Sqrt)

            # Wo projection + residual -> y, LN1
            y = acts16.tile([128, TT, H], f16, tag="act16", name=f"y_{l}")
            yT = acts16.tile([128, HC, S], f16, tag="act16", name=f"yT_{l}")
            g1_bc, b1_bc = _ln_bcast(nc, pools, g1_d[l], b1_d[l], ln1_aff)
            woblk = wvo.tile([128, HC, 768], f16, tag="wvo", name="wo_blk")
            nc.sync.dma_start(woblk[:], wo_d[l])
            if o_bias:
                bo_row = brow_p.tile([1, H], f16, tag="br", name="bo_row")
                nc.sync.dma_start(bo_row[:], bo_d[l][None, :])
            s4y = vec.tile([128, 4], f32, tag="v", name=f"s4y_{l}")
            for tt in range(TT):
                acc = psm2.tile([128, 2, S], f32, tag="m2", name="wo_acc")
                for n in range(2):
                    if o_bias:
                        nc.tensor.matmul(acc[:, n, 0:384],
                                         lhsT=ones_sb[0:1, 0:128],
                                         rhs=bo_row[0:1, n * 384:(n + 1) * 384],
                                         start=True, stop=False)
                    for jc in range(HC):
                        nc.tensor.matmul(
                            acc[:, n, 0:384],
                            lhsT=attnT[:, jc, tt * 128:tt * 128 + 128],
                            rhs=woblk[:, jc, n * 384:(n + 1) * 384],
                            start=(not o_bias and jc == 0), stop=(jc == HC - 1))
                nc.vector.scalar_tensor_tensor(
                    out=y[:, tt, :], in0=acc[:, :, 0:384], scalar=1.0,
                    in1=x[:, tt, :], op0=OP.mult, op1=OP.add,
                    accum_out=s4y[:, tt:tt + 1])
                if tt == 1:
                    _ln_pair(nc, pools, y, s4y, (0, 1), g1_bc, b1_bc)
                    _transpose_half(nc, pools, y, yT, ident, (0, 1))
            _ln_pair(nc, pools, y, s4y, (2, 3), g1_bc, b1_bc)
            _transpose_half(nc, pools, y, yT, ident, (2, 3))
            if dbg and l == 0 and dbg == "y":
                _dbg_dump(nc, dbg_d, y[:], TT * H)

            # FFN up: h1T = gelu(yT @ Wi + bi), [128, FC//2, 2, S] f16
            h1T = h1p.tile([128, FC // 2, 2, S], f16, tag="h1", name=f"h1T_{l}")
            for p in range(FC // 2):
                wiblk = wip.tile([128, 2, HC, 128], f16, tag="wi", name="wi_blk")
                nc.sync.dma_start(wiblk[:], wi_d[l, p])
                ph = psm2.tile([128, 2, S], f32, tag="m2", name="ph")
                for q in range(2):
                    for ic in range(HC):
                        nc.tensor.matmul(ph[:, q, :], lhsT=wiblk[:, q, ic, :],
                                         rhs=yT[:, ic, :],
                                         start=(ic == 0), stop=(ic == HC - 1))
                if i_bias:
                    for q in range(2):
                        fc = 2 * p + q
                        bi_sl = vec.tile([128, 1], f32, tag="v", name="bi_sl")
                        nc.sync.dma_start(
                            bi_sl[:], bi_d[l, fc * 128:(fc + 1) * 128][:, None])
                        nc.scalar.activation(h1T[:, p, q, :], ph[:, q, :],
                                             AF.Gelu, bias=bi_sl[:])
                else:
                    nc.scalar.activation(h1T[:, p], ph[:], AF.Gelu)

            if dbg and l == 0 and dbg == "h1":
                _dbg_dump(nc, dbg_d, h1T[:], FC * S)

            _act_hoist(nc, pools, AF.Sqrt)

            # FFN down + residual -> ffnout, LN2; waves of 2 token tiles
            ffnout = acts16.tile([128, TT, H], f16, tag="act16", name=f"ffn_{l}")
            g2_bc, b2_bc = _ln_bcast(nc, pools, g2_d[l], b2_d[l], ln2_aff)
            if d_bias:
                bd_row = brow_p.tile([1, H], f16, tag="br", name="bd_row")
                nc.sync.dma_start(bd_row[:], bd_d[l][None, :])
            s4f = vec.tile([128, 4], f32, tag="v", name=f"s4f_{l}")
            last = l == n_layers - 1
            if not last:
                xT_next = acts16.tile([128, HC, S], f16, tag="act16",
                                      name=f"xT_{l + 1}")
            for wave in range(2):
                tts = (0, 1) if wave == 0 else (2, 3)
                accs = {}
                for tt in tts:
                    acc = psm2.tile([128, 2, S], f32, tag="m2", name=f"fd_acc{tt}")
                    if d_bias:
                        for n in range(2):
                            nc.tensor.matmul(
                                acc[:, n, 0:384], lhsT=ones_sb[0:1, 0:128],
                                rhs=bd_row[0:1, n * 384:(n + 1) * 384],
                                start=True, stop=False)
                    accs[tt] = acc
                for fp in range(FC // 4):
                    wdblk = wdp.tile([128, 4, H], f16, tag="wd", name="wd_blk")
                    nc.sync.dma_start(wdblk[:], wd_d[l, fp])
                    if fp == FC // 4 - 1:
                        # tt-major on the last block so the tt evictions and
                        # their LN chains stagger instead of arriving together
                        for tt in tts:
                            for k4 in range(4):
                                fc = 4 * fp + k4
                                for n in range(2):
                                    nc.tensor.matmul(
                                        accs[tt][:, n, 0:384],
                                        lhsT=h1T[:, fc // 2, fc % 2,
                                                 tt * 128:tt * 128 + 128],
                                        rhs=wdblk[:, k4, n * 384:(n + 1) * 384],
                                        start=False, stop=(fc == FC - 1))
                        continue
                    for k4 in range(4):
                        fc = 4 * fp + k4
                        for tt in tts:
                            for n in range(2):
                                nc.tensor.matmul(
                                    accs[tt][:, n, 0:384],
                                    lhsT=h1T[:, fc // 2, fc % 2,
                                             tt * 128:tt * 128 + 128],
                                    rhs=wdblk[:, k4, n * 384:(n + 1) * 384],
                                    start=(not d_bias and fc == 0),
                                    stop=False)
                for tt in tts:
                    nc.vector.scalar_tensor_tensor(
                        out=ffnout[:, tt, :], in0=accs[tt][:, :, 0:384],
                        scalar=1.0, in1=y[:, tt, :], op0=OP.mult, op1=OP.add,
                        accum_out=s4f[:, tt:tt + 1])
                _ln_pair(nc, pools, ffnout, s4f, tts, g2_bc, b2_bc)
                if not last:
                    _transpose_half(nc, pools, ffnout, xT_next, ident, tts)
            _act_hoist(nc, pools, AF.Exp)
            x = ffnout
            if not last:
                xT = xT_next

        for tt in range(TT):
            xo = scratch.tile([128, H], f32, tag="sc", name="out_cast")
            nc.vector.tensor_copy(xo[:], x[:, tt, :])
            nc.sync.dma_start(out_d[tt * 128:(tt + 1) * 128, :], xo[:])

    nc.compile()
    return nc


def _prep_inputs(inputs, b):
    f = np.float32
    h = np.float16
    Wq, Wk, Wv, Wo, Wi, Wd = (np.asarray(inputs[k], f)
                              for k in ("Wq", "Wk", "Wv", "Wo", "Wi", "Wd"))
    # [L, H, H] -> [l, j, k, ic, m]: out-chunk j rows=contraction chunk ic
    Wq16 = np.ascontiguousarray(
        Wq.reshape(L, HC, 128, HC, 128).transpose(0, 3, 2, 1, 4)).astype(h)
    Wk16 = np.ascontiguousarray(
        Wk.reshape(L, HC, 128, HC, 128).transpose(0, 3, 2, 1, 4)).astype(h)
    # [L, H, H] -> [l, k, ic, col]
    Wv16 = np.ascontiguousarray(
        Wv.reshape(L, HC, 128, H).transpose(0, 2, 1, 3)).astype(h)
    Wo16 = np.ascontiguousarray(
        Wo.reshape(L, HC, 128, H).transpose(0, 2, 1, 3)).astype(h)
    # [L, H, I] -> [l, p, k, q, ic, m]
    Wi16 = np.ascontiguousarray(
        Wi.reshape(L, HC, 128, FC // 2, 2, 128).transpose(0, 3, 2, 4, 1, 5)
    ).astype(h)
    # [L, I, H] -> [l, fp, k, k4, col]: quads of 128-row blocks for batched DMA
    Wd16 = np.ascontiguousarray(
        Wd.reshape(L, FC // 4, 4, 128, H).transpose(0, 1, 3, 2, 4)).astype(h)
    mask = np.asarray(inputs["input_mask"], f)
    tti = np.asarray(inputs["token_type_ids"], np.int32)
    flags = dict(
        qk_bias=bool(np.any(np.asarray(inputs["bq"])) or np.any(np.asarray(inputs["bk"]))),
        v_bias=bool(np.any(np.asarray(inputs["bv"]))),
        o_bias=bool(np.any(np.asarray(inputs["bo"]))),
        i_bias=bool(np.any(np.asarray(inputs["bi"]))),
        d_bias=bool(np.any(np.asarray(inputs["bd"]))),
        ln1_aff=bool(np.any(np.asarray(inputs["ln1_g"]) != 1.0) or
                     np.any(np.asarray(inputs["ln1_b"]))),
        ln2_aff=bool(np.any(np.asarray(inputs["ln2_g"]) != 1.0) or
                     np.any(np.asarray(inputs["ln2_b"]))),
        emb_aff=bool(np.any(np.asarray(inputs["emb_ln_g"]) != 1.0) or
                     np.any(np.asarray(inputs["emb_ln_b"]))),
        use_mask=bool(np.any(mask != 1.0)),
        use_type=bool(np.any(tti != 0)),
    )
    pos_eff = np.asarray(inputs["pos_emb"], f)[:S]
    if not flags["use_type"]:
        pos_eff = pos_eff + np.asarray(inputs["type_emb"], f)[int(tti.flat[0])][None, :]
    shared = dict(
        tok_emb=np.asarray(inputs["tok_emb"], f),
        pos_emb=pos_eff,
        type_emb=np.asarray(inputs["type_emb"], f),
        emb_g=np.asarray(inputs["emb_ln_g"], f),
        emb_b=np.asarray(inputs["emb_ln_b"], f),
        Wq16=Wq16, Wk16=Wk16, Wv16=Wv16, Wo16=Wo16, Wi16=Wi16, Wd16=Wd16,
        bq=np.asarray(inputs["bq"], f), bk=np.asarray(inputs["bk"], f),
        bv=np.asarray(inputs["bv"], f),
        bo=np.asarray(inputs["bo"], f).astype(h),
        bi=np.asarray(inputs["bi"], f),
        bd=np.asarray(inputs["bd"], f).astype(h),
        ln1_g=np.asarray(inputs["ln1_g"], f), ln1_b=np.asarray(inputs["ln1_b"], f),
        ln2_g=np.asarray(inputs["ln2_g"], f), ln2_b=np.asarray(inputs["ln2_b"], f),
        ones16=np.ones((128, 128), h),
        ident=np.eye(128, dtype=h),
    )
    in_maps = []
    ids = np.asarray(inputs["input_ids"], np.int32)
    for c in range(b):
        m = dict(shared)
        m["ids"] = np.ascontiguousarray(ids[c])
        m["tti"] = np.ascontiguousarray(tti[c])
        m["mb"] = np.ascontiguousarray((1.0 - mask[c]) * -10000.0)
        in_maps.append(m)
    return in_maps, flags


def kernel(**inputs):
    global LAST_EXEC_TIME_NS, LAST_RESULT
    n_layers = int(os.environ.get("BERT_LAYERS", L))
    trace = bool(os.environ.get("BERT_TRACE"))
    in_maps, flags = _prep_inputs(inputs, B)
    nc = build(n_layers, flags)
    res = bass_utils.run_bass_kernel_spmd(
        nc, in_maps, core_ids=list(range(B)), trace=trace)
    LAST_EXEC_TIME_NS = res.exec_time_ns
    LAST_RESULT = res
    out = np.stack([res.results[c]["out"] for c in range(B)])
    return out.astype(np.float32)


# revision 59
# speedup vs baseline: 1.0002x; 1.0002x over previous
"""BERT-base forward on 8 Trainium2 NeuronCores, data-parallel over batch.

Each core runs the full 12-layer model on one batch element (512 tokens).
v2: all matmul operands in fp16 (1 cyc/row on PE, FWL weight loads, half the
weight DMA of f32r; rel-err ~6e-4 vs fp32 reference). Residual stream stays
fp32 token-major; hidden-major operands (xT/yT/QT/KT/attnT/h1T) are fp16.

Key structure per layer / core (SBUF tiles are [128 partitions, ...]):
  x token-major f32 [128, TT, H] -> PE-transpose (f32r) -> xT f16 [128, HC, S]
  QT/KT f16 via 6x6 128-blocked matmuls, evicted from 2-bank PSUM pairs.
  V token-major f16 [128, TT, H].
  Attention per head pair c: scores for heads (2c, 2c+1) are row-packed
  (tile_position rows 0:64 / 64:128) into one 2-bank PSUM tile per k-chunk;
  ONE Exp per [128, 2*S] tile (halves ACT op overhead); denominators and
  O^T=V'expS are column-packed pairs (out partitions 0:64 / 64:128, separate
  banks) so the two heads' matmuls run concurrently in the PE array.
  Wo/FFN-down accumulate (n=0,1) halves into one 2-bank PSUM tile; eviction is
  a single scalar_tensor_tensor that also adds the residual AND produces the
  LN row-sum via accum_out. LN variance via ACT Square+accum; rstd via
  bit-trick + 2 Newton steps on DVE (no Sqrt -> no ACT table switch); the
  normalize runs on GpSimd to unload DVE. Exp/Gelu are the only table sets.

Work that is provably a no-op for the given inputs (zero biases, unit gammas,
zero betas, all-ones mask) is skipped at build time; general paths stay
available and are selected per-input on the host.
"""
import os
import numpy as np
import ml_dtypes
from contextlib import ExitStack

import concourse.bass as bass
import concourse.tile as tile
from concourse import bacc, mybir
from concourse import bass_utils

f32 = mybir.dt.float32
f32r = mybir.dt.float32r
f16 = mybir.dt.float16
i32 = mybir.dt.int32
AF = mybir.ActivationFunctionType
OP = mybir.AluOpType
AX = mybir.AxisListType

V, H, L, NH, I, P, B, S = 30000, 768, 12, 12, 3072, 512, 8, 512
D = H // NH          # 64
HC = H // 128        # 6 hidden chunks
FC = I // 128        # 24 ffn chunks
TT = S // 128        # 4 token tiles
LN_EPS = 1e-3
MAGIC = 0x5F3759DF

LAST_EXEC_TIME_NS = None
LAST_RESULT = None


def _act_hoist(nc, pools, func):
    """Tiny ACT op that forces the table set for `func` to load here (off the
    critical path) instead of right before the first real use."""
    one = pools["act_one"]
    j = pools["vec"].tile([128, 1], f32, tag="v", name="act_pre")
    nc.scalar.activation(j[:], one[:], func)


def _ln_pair(nc, pools, z, s4, tts, g_bc, b_bc):
    """LayerNorm (in place, over hidden) for token tiles `tts` of z.

    z [128, TT, H] f16; s4 [128, 4] holds per-tile row sums (cols = tt) already
    accumulated by the evictions. Processing tile-pairs right after their
    evictions keeps this chain off the critical path of the following phase.
    rstd comes from ACT Sqrt + DVE reciprocal; the sqrt table set is preloaded
    off the critical path via _act_hoist.
    """
    vec, scratch = pools["vec"], pools["scratch"]
    w = len(tts)
    t0 = tts[0]
    sp = s4[:, t0:t0 + w]
    ssq = vec.tile([128, w], f32, tag="v", name="ln_ssq")
    for i, tt in enumerate(tts):
        sq = scratch.tile([128, H], f32, tag="sc", name="ln_sq")
        nc.scalar.activation(sq[:], z[:, tt, :], AF.Square,
                             accum_out=ssq[:, i:i + 1])
    b2 = vec.tile([128, w], f32, tag="v", name="ln_b2")
    nc.vector.scalar_tensor_tensor(out=b2[:], in0=sp,
                                   scalar=float(-1.0 / (H * H)), in1=sp,
                                   op0=OP.mult, op1=OP.mult)
    nc.vector.tensor_scalar(out=b2[:], in0=b2[:], scalar1=float(LN_EPS),
                            scalar2=None, op0=OP.add)
    sd = vec.tile([128, w], f32, tag="v", name="ln_sd")
    for i in range(w):
        nc.scalar.activation(sd[:, i:i + 1], ssq[:, i:i + 1], AF.Sqrt,
                             bias=b2[:, i:i + 1], scale=float(1.0 / H))
    r = vec.tile([128, w], f32, tag="v", name="ln_r")
    nc.vector.reciprocal(r[:], sd[:])
    mr = vec.tile([128, w], f32, tag="v", name="ln_mr")
    nc.vector.scalar_tensor_tensor(out=mr[:], in0=sp,
                                   scalar=float(-1.0 / H), in1=r[:],
                                   op0=OP.mult, op1=OP.mult)
    for i, tt in enumerate(tts):
        eng = nc.vector if (tt % 2 == 0) else nc.gpsimd
        eng.tensor_scalar(out=z[:, tt, :], in0=z[:, tt, :],
                          scalar1=r[:, i:i + 1], scalar2=mr[:, i:i + 1],
                          op0=OP.mult, op1=OP.add)
        if g_bc is not None:
            nc.vector.tensor_tensor(out=z[:, tt, :], in0=z[:, tt, :],
                                    in1=g_bc[:], op=OP.mult)
        if b_bc is not None:
            nc.vector.tensor_tensor(out=z[:, tt, :], in0=z[:, tt, :],
                                    in1=b_bc[:], op=OP.add)


def _ln_apply(nc, pools, z, s4, g_bc, b_bc):
    _ln_pair(nc, pools, z, s4, (0, 1), g_bc, b_bc)
    _ln_pair(nc, pools, z, s4, (2, 3), g_bc, b_bc)


def _ln_bcast(nc, pools, g_row, b_row, affine):
    if not affine:
        return None, None
    gb = pools["gb"]
    g_bc = gb.tile([128, H], f32, tag="gb", name="g_bc")
    nc.sync.dma_start(g_bc[:], g_row[None, :].partition_broadcast(128))
    b_bc = gb.tile([128, H], f32, tag="gb", name="b_bc")
    nc.sync.dma_start(b_bc[:], b_row[None, :].partition_broadcast(128))
    return g_bc, b_bc


def _transpose_half(nc, pools, src, dst, ident, tts):
    """Transpose token tiles `tts` (a (0,1) or (2,3) pair) of token-major src
    [128, TT, H] f16 into the matching column half of hidden-major dst
    [128, HC, S] f16. Split by halves so the (0,1) half runs as soon as its
    LayerNorm pair lands, under the tail of the producing phase."""
    psm1 = pools["psm1"]
    t0 = tts[0]
    for c in range(HC):
        tp = psm1.tile([128, 256], f32, tag="m1", name="tp")
        for i, tt in enumerate(tts):
            # out = src_block.T via plain matmul with identity as the moving
            # operand: out[m, n] = sum_p src[p, m] * I[p, n] = src[n, m].
            nc.tensor.matmul(tp[:, i * 128:(i + 1) * 128],
                             lhsT=src[:, tt, c * 128:c * 128 + 128],
                             rhs=ident[:], start=True, stop=True)
        # evict on ACT (Copy is in every table set): DVE is the serializer on
        # the LN->transpose tail while ACT is idle there
        nc.scalar.copy(dst[:, c, t0 * 128:t0 * 128 + 256], tp[:])


def _transpose_into(nc, pools, src, dst, ident):
    _transpose_half(nc, pools, src, dst, ident, (0, 1))
    _transpose_half(nc, pools, src, dst, ident, (2, 3))


def _dbg_dump(nc, dbg_d, src_ap, n):
    """DMA an SBUF view with free size n to the raw debug output."""
    nc.sync.dma_start(dbg_d[:, 0:n], src_ap)


def build(n_layers=L, flags=None):
    fl = flags or {}
    qk_bias = fl.get("qk_bias", True)
    v_bias = fl.get("v_bias", True)
    o_bias = fl.get("o_bias", True)
    i_bias = fl.get("i_bias", True)
    d_bias = fl.get("d_bias", True)
    ln1_aff = fl.get("ln1_aff", True)
    ln2_aff = fl.get("ln2_aff", True)
    emb_aff = fl.get("emb_aff", True)
    use_mask = fl.get("use_mask", True)
    use_type = fl.get("use_type", True)

    nc = bacc.Bacc("TRN2", target_bir_lowering=False, debug=False, num_devices=8)

    dt_in = lambda n, s, d: nc.dram_tensor(n, s, d, kind="ExternalInput").ap()
    ids_d = dt_in("ids", [S], i32)
    tti_d = dt_in("tti", [S], i32)
    mb_d = dt_in("mb", [S], f32)
    tok_d = dt_in("tok_emb", [V, H], f32)
    pos_d = dt_in("pos_emb", [S, H], f32)
    typ_d = dt_in("type_emb", [2, H], f32)
    eg_d = dt_in("emb_g", [H], f32)
    eb_d = dt_in("emb_b", [H], f32)
    wq_d = dt_in("Wq16", [L, HC, 128, HC, 128], f16)
    wk_d = dt_in("Wk16", [L, HC, 128, HC, 128], f16)
    wv_d = dt_in("Wv16", [L, 128, HC, 768], f16)
    wo_d = dt_in("Wo16", [L, 128, HC, 768], f16)
    wi_d = dt_in("Wi16", [L, FC // 2, 128, 2, HC, 128], f16)
    wd_d = dt_in("Wd16", [L, FC // 4, 128, 4, H], f16)
    bq_d = dt_in("bq", [L, H], f32)
    bk_d = dt_in("bk", [L, H], f32)
    bv_d = dt_in("bv", [L, H], f32)
    bo_d = dt_in("bo", [L, H], f16)
    bi_d = dt_in("bi", [L, I], f32)
    bd_d = dt_in("bd", [L, H], f16)
    g1_d = dt_in("ln1_g", [L, H], f32)
    b1_d = dt_in("ln1_b", [L, H], f32)
    g2_d = dt_in("ln2_g", [L, H], f32)
    b2_d = dt_in("ln2_b", [L, H], f32)
    ones_d = dt_in("ones16", [128, 128], f16)
    ident_d = dt_in("ident", [128, 128], f16)
    out_d = nc.dram_tensor("out", [S, H], f32, kind="ExternalOutput").ap()
    dbg = os.environ.get("BERT_DBG")
    dbg_d = None
    if dbg:
        dbg_dt = f32 if dbg in ("dnav",) else f16
        dbg_d = nc.dram_tensor("dbg", [128, FC * S], dbg_dt,
                               kind="ExternalOutput").ap()

    with tile.TileContext(nc) as tc, ExitStack() as ctx:
        acts16 = ctx.enter_context(tc.tile_pool(name="acts16", bufs=12))
        h1p = ctx.enter_context(tc.tile_pool(name="h1p", bufs=1))
        wqk = ctx.enter_context(tc.tile_pool(name="wqk", bufs=6))
        wvo = ctx.enter_context(tc.tile_pool(name="wvo", bufs=2))
        wip = ctx.enter_context(tc.tile_pool(name="wip", bufs=3))
        wdp = ctx.enter_context(tc.tile_pool(name="wdp", bufs=3))
        esp = ctx.enter_context(tc.tile_pool(name="esp", bufs=14))
        bcp = ctx.enter_context(tc.tile_pool(name="bcp", bufs=4))
        gb = ctx.enter_context(tc.tile_pool(name="gb", bufs=2))
        scratch = ctx.enter_context(tc.tile_pool(name="scratch", bufs=2))
        vec = ctx.enter_context(tc.tile_pool(name="vec", bufs=28))
        brow_p = ctx.enter_context(tc.tile_pool(name="brow_p", bufs=2))
        const = ctx.enter_context(tc.tile_pool(name="const", bufs=1))
        psm1 = ctx.enter_context(tc.tile_pool(name="psm1", bufs=2, space="PSUM"))
        psm2 = ctx.enter_context(tc.tile_pool(name="psm2", bufs=3, space="PSUM"))

        pools = dict(gb=gb, vec=vec, scratch=scratch, psm1=psm1, psm2=psm2)

        # constants
        ones_sb = const.tile([128, 128], f16, tag="ones", name="ones_sb")
        nc.sync.dma_start(ones_sb[:], ones_d[:])
        ident = const.tile([128, 128], f16, tag="ident", name="ident")
        nc.sync.dma_start(ident[:], ident_d[:])
        act_one = const.tile([128, 1], f32, tag="aone", name="act_one")
        nc.vector.memset(act_one[:], 1.0)
        pools["act_one"] = act_one
        ids_sb = const.tile([128, TT], i32, tag="ids", name="ids_sb")
        nc.sync.dma_start(ids_sb[:], ids_d.rearrange("(t p) -> p t", p=128))
        if use_type:
            tti_sb = const.tile([128, TT], i32, tag="tti", name="tti_sb")
            nc.sync.dma_start(tti_sb[:], tti_d.rearrange("(t p) -> p t", p=128))
        if use_mask:
            mb_sb = const.tile([128, TT], f32, tag="mb", name="mb_sb")
            nc.sync.dma_start(mb_sb[:], mb_d.rearrange("(t p) -> p t", p=128))

        # ---- embedding ----
        x = acts16.tile([128, TT, H], f16, tag="act16", name="x_emb")
        eg_bc, eb_bc = _ln_bcast(nc, pools, eg_d, eb_d, emb_aff)
        s4e = vec.tile([128, 4], f32, tag="v", name="s4_emb")
        for tt in range(TT):
            xg = scratch.tile([128, H], f32, tag="sc", name="emb_gather")
            nc.gpsimd.indirect_dma_start(
                out=xg[:], out_offset=None, in_=tok_d[:],
                in_offset=bass.IndirectOffsetOnAxis(ap=ids_sb[:, tt:tt + 1], axis=0))
            if use_type:
                tmp_t = gb.tile([128, H], f32, tag="gb", name="emb_tmp")
                nc.gpsimd.indirect_dma_start(
                    out=tmp_t[:], out_offset=None, in_=typ_d[:],
                    in_offset=bass.IndirectOffsetOnAxis(ap=tti_sb[:, tt:tt + 1], axis=0))
                nc.vector.tensor_tensor(out=xg[:], in0=xg[:], in1=tmp_t[:],
                                        op=OP.add)
            tmp_p = gb.tile([128, H], f32, tag="gb", name="emb_pos")
            nc.sync.dma_start(tmp_p[:], pos_d[tt * 128:(tt + 1) * 128, :])
            nc.vector.scalar_tensor_tensor(out=x[:, tt, :], in0=tmp_p[:],
                                           scalar=1.0, in1=xg[:],
                                           op0=OP.mult, op1=OP.add,
                                           accum_out=s4e[:, tt:tt + 1])
        _ln_apply(nc, pools, x, s4e, eg_bc, eb_bc)
        xT = acts16.tile([128, HC, S], f16, tag="act16", name="xT_emb")
        _transpose_into(nc, pools, x, xT, ident)

        # ---- layers ----
        for l in range(n_layers):
            # V token-major f16 [128, TT, H] (needed by the per-chunk attention)
            Vt = acts16.tile([128, TT, H], f16, tag="act16", name=f"V_{l}")
            wvblk = wvo.tile([128, HC, 768], f16, tag="wvo", name="wv_blk")
            nc.sync.dma_start(wvblk[:], wv_d[l])
            for tt in range(TT):
                pv = psm2.tile([128, 2, S], f32, tag="m2", name="pv")
                for n in range(2):
                    for ic in range(HC):
                        nc.tensor.matmul(
                            pv[:, n, 0:384],
                            lhsT=xT[:, ic, tt * 128:tt * 128 + 128],
                            rhs=wvblk[:, ic, n * 384:(n + 1) * 384],
                            start=(ic == 0), stop=(ic == HC - 1))
                nc.vector.tensor_copy(Vt[:, tt, :], pv[:, :, 0:384])

            # Q^T/K^T production interleaved with per-head-pair attention:
            # after chunk pair jj of both Q and K is evicted, attention for
            # head pairs c = 2jj, 2jj+1 runs (its exps keep ACT busy while the
            # PE streams the next jj's projections).
            QT = acts16.tile([128, HC, S], f16, tag="act16", name=f"QT_{l}")
            KT = acts16.tile([128, HC, S], f16, tag="act16", name=f"KT_{l}")
            attnT = acts16.tile([128, HC, S], f16, tag="act16", name=f"attnT_{l}")
            for jj in range(HC // 2):
                for dst, w_d, b_d in ((QT, wq_d, bq_d), (KT, wk_d, bk_d)):
                    acc = psm2.tile([128, 2, S], f32, tag="m2", name="qk_acc")
                    for j2 in range(2):
                        j = 2 * jj + j2
                        wblk = wqk.tile([128, HC, 128], f16, tag="wqk",
                                        name="wqk_blk")
                        nc.sync.dma_start(wblk[:], w_d[l, j])
                        for ic in range(HC):
                            nc.tensor.matmul(acc[:, j2, :], lhsT=wblk[:, ic, :],
                                             rhs=xT[:, ic, :],
                                             start=(ic == 0), stop=(ic == HC - 1))
                    if qk_bias:
                        for j2 in range(2):
                            j = 2 * jj + j2
                            b_sl = vec.tile([128, 1], f32, tag="v", name="bqk_sl")
                            nc.sync.dma_start(
                                b_sl[:], b_d[l, j * 128:(j + 1) * 128][:, None])
                            nc.scalar.activation(dst[:, j, :], acc[:, j2, :],
                                                 AF.Identity, bias=b_sl[:])
                    else:
                        nc.vector.tensor_copy(dst[:, 2 * jj:2 * jj + 2, :], acc[:])
                for c in (2 * jj, 2 * jj + 1):
                    # dn/av accumulate column-packed (head A rows 0:64, head B
                    # rows 64:128, concurrent via col groups). The PE stream is
                    # software-pipelined at k-chunk granularity: scores run two
                    # chunks ahead of the dn/av consumers so the PE never
                    # head-of-line blocks on an Exp result.
                    dn = psm1.tile([128, S], f32, tag="m1", name="dn")
                    av = psm1.tile([128, S], f32, tag="m1", name="av")
                    es = [None] * TT

                    def emit_scores(kc, c=c, es=es):
                        sp = psm2.tile([128, 2, S], f32, tag="m2", name="sp")
                        for hh in range(2):
                            r0 = 64 * hh
                            nc.tensor.matmul(
                                sp[:, hh, :],
                                lhsT=KT[r0:r0 + 64, c, kc * 128:kc * 128 + 128],
                                rhs=QT[r0:r0 + 64, c, :],
                                start=True, stop=True)
                        e = esp.tile([128, 2, S], f16, tag="es", name=f"e{kc}")
                        if use_mask:
                            mbias = mb_sb[:, kc:kc + 1]
                            for hh in range(2):
                                nc.scalar.activation(e[:, hh, :], sp[:, hh, :],
                                                     AF.Exp, bias=mbias,
                                                     scale=0.125)
                        else:
                            nc.scalar.activation(e[:], sp[:], AF.Exp, scale=0.125)
                        es[kc] = e

                    def emit_dnav(kc, c=c, es=es, dn=dn, av=av):
                        for hh in range(2):
                            nc.tensor.matmul(dn[64 * hh:64 * hh + 64, :],
                                             lhsT=ones_sb[:, 0:64],
                                             rhs=es[kc][:, hh, :],
                                             start=(kc == 0), stop=(kc == TT - 1))
                        for hh in range(2):
                            nc.tensor.matmul(
                                av[64 * hh:64 * hh + 64, :],
                                lhsT=Vt[:, kc,
                                        (2 * c + hh) * D:(2 * c + hh + 1) * D],
                                rhs=es[kc][:, hh, :],
                                start=(kc == 0), stop=(kc == TT - 1))

                    emit_scores(0)
                    emit_scores(1)
                    emit_dnav(0)
                    emit_scores(2)
                    emit_dnav(1)
                    emit_scores(3)
                    emit_dnav(2)
                    emit_dnav(3)
                    bct = bcp.tile([128, S], f32, tag="bc", name="bct")
                    nc.vector.reciprocal_approx_fast(out=bct[:], in_=dn[:])
                    nc.vector.tensor_tensor(out=attnT[:, c, :], in0=av[:],
                                            in1=bct[:], op=OP.mult)
                    if v_bias:
                        bv_sl = vec.tile([128, 1], f32, tag="v", name="bv_sl")
                        nc.sync.dma_start(
                            bv_sl[:], bv_d[l, 2 * c * D:(2 * c + 2) * D][:, None])
                        nc.vector.tensor_scalar(
                            out=attnT[:, c, :], in0=attnT[:, c, :].bitcast(f16),
                            scalar1=bv_sl[:], scalar2=None, op0=OP.add)

            if dbg and l == 0 and dbg in ("xt", "qt", "kt"):
                src = {"xt": xT, "qt": QT, "kt": KT}[dbg]
                _dbg_dump(nc, dbg_d, src[:], HC * S)
            if dbg and l == 0 and dbg in ("vt", "attnt"):
                src = {"vt": Vt, "attnt": attnT}[dbg]
                _dbg_dump(nc, dbg_d, src[:], TT * H)

            _act_hoist(nc, pools, AF.Sqrt)

            # Wo projection + residual -> y, LN1
            y = acts16.tile([128, TT, H], f16, tag="act16", name=f"y_{l}")
            yT = acts16.tile([128, HC, S], f16, tag="act16", name=f"yT_{l}")
            g1_bc, b1_bc = _ln_bcast(nc, pools, g1_d[l], b1_d[l], ln1_aff)
            woblk = wvo.tile([128, HC, 768], f16, tag="wvo", name="wo_blk")
            nc.sync.dma_start(woblk[:], wo_d[l])
            if o_bias:
                bo_row = brow_p.tile([1, H], f16, tag="br", name="bo_row")
                nc.sync.dma_start(bo_row[:], bo_d[l][None, :])
            s4y = vec.tile([128, 4], f32, tag="v", name=f"s4y_{l}")
            for tt in range(TT):
                acc = psm2.tile([128, 2, S], f32, tag="m2", name="wo_acc")
                for n in range(2):
                    if o_bias:
                        nc.tensor.matmul(acc[:, n, 0:384],
                                         lhsT=ones_sb[0:1, 0:128],
                                         rhs=bo_row[0:1, n * 384:(n + 1) * 384],
                                         start=True, stop=False)
                    for jc in range(HC):
                        nc.tensor.matmul(
                            acc[:, n, 0:384],
                            lhsT=attnT[:, jc, tt * 128:tt * 128 + 128],
                            rhs=woblk[:, jc, n * 384:(n + 1) * 384],
                            start=(not o_bias and jc == 0), stop=(jc == HC - 1))
                nc.vector.scalar_tensor_tensor(
                    out=y[:, tt, :], in0=acc[:, :, 0:384], scalar=1.0,
                    in1=x[:, tt, :], op0=OP.mult, op1=OP.add,
                    accum_out=s4y[:, tt:tt + 1])
                if tt == 1:
                    _ln_pair(nc, pools, y, s4y, (0, 1), g1_bc, b1_bc)
                    _transpose_half(nc, pools, y, yT, ident, (0, 1))
            _ln_pair(nc, pools, y, s4y, (2, 3), g1_bc, b1_bc)
            _transpose_half(nc, pools, y, yT, ident, (2, 3))
            if dbg and l == 0 and dbg == "y":
                _dbg_dump(nc, dbg_d, y[:], TT * H)

            # FFN up: h1T = gelu(yT @ Wi + bi), [128, FC//2, 2, S] f16
            h1T = h1p.tile([128, FC // 2, 2, S], f16, tag="h1", name=f"h1T_{l}")
            for p in range(FC // 2):
                wiblk = wip.tile([128, 2, HC, 128], f16, tag="wi", name="wi_blk")
                nc.sync.dma_start(wiblk[:], wi_d[l, p])
                ph = psm2.tile([128, 2, S], f32, tag="m2", name="ph")
                for q in range(2):
                    for ic in range(HC):
                        nc.tensor.matmul(ph[:, q, :], lhsT=wiblk[:, q, ic, :],
                                         rhs=yT[:, ic, :],
                                         start=(ic == 0), stop=(ic == HC - 1))
                if i_bias:
                    for q in range(2):
                        fc = 2 * p + q
                        bi_sl = vec.tile([128, 1], f32, tag="v", name="bi_sl")
                        nc.sync.dma_start(
                            bi_sl[:], bi_d[l, fc * 128:(fc + 1) * 128][:, None])
                        nc.scalar.activation(h1T[:, p, q, :], ph[:, q, :],
                                             AF.Gelu, bias=bi_sl[:])
                else:
                    nc.scalar.activation(h1T[:, p], ph[:], AF.Gelu)

            if dbg and l == 0 and dbg == "h1":
                _dbg_dump(nc, dbg_d, h1T[:], FC * S)

            _act_hoist(nc, pools, AF.Sqrt)

            # FFN down + residual -> ffnout, LN2; waves of 2 token tiles
            ffnout = acts16.tile([128, TT, H], f16, tag="act16", name=f"ffn_{l}")
            g2_bc, b2_bc = _ln_bcast(nc, pools, g2_d[l], b2_d[l], ln2_aff)
            if d_bias:
                bd_row = brow_p.tile([1, H], f16, tag="br", name="bd_row")
                nc.sync.dma_start(bd_row[:], bd_d[l][None, :])
            s4f = vec.tile([128, 4], f32, tag="v", name=f"s4f_{l}")
            last = l == n_layers - 1
            if not last:
                xT_next = acts16.tile([128, HC, S], f16, tag="act16",
                                      name=f"xT_{l + 1}")
            for wave in range(2):
                tts = (0, 1) if wave == 0 else (2, 3)
                accs = {}
                for tt in tts:
                    acc = psm2.tile([128, 2, S], f32, tag="m2", name=f"fd_acc{tt}")
                    if d_bias:
                        for n in range(2):
                            nc.tensor.matmul(
                                acc[:, n, 0:384], lhsT=ones_sb[0:1, 0:128],
                                rhs=bd_row[0:1, n * 384:(n + 1) * 384],
                                start=True, stop=False)
                    accs[tt] = acc
                for fp in range(FC // 4):
                    wdblk = wdp.tile([128, 4, H], f16, tag="wd", name="wd_blk")
                    nc.sync.dma_start(wdblk[:], wd_d[l, fp])
                    if fp == FC // 4 - 1:
                        # tt-major on the last block so the tt evictions and
                        # their LN chains stagger instead of arriving together
                        for tt in tts:
                            for k4 in range(4):
                                fc = 4 * fp + k4
                                for n in range(2):
                                    nc.tensor.matmul(
                                        accs[tt][:, n, 0:384],
                                        lhsT=h1T[:, fc // 2, fc % 2,
                                                 tt * 128:tt * 128 + 128],
                                        rhs=wdblk[:, k4, n * 384:(n + 1) * 384],
                                        start=False, stop=(fc == FC - 1))
                        continue
                    for k4 in range(4):
                        fc = 4 * fp + k4
                        for tt in tts:
                            for n in range(2):
                                nc.tensor.matmul(
                                    accs[tt][:, n, 0:384],
                                    lhsT=h1T[:, fc // 2, fc % 2,
                                             tt * 128:tt * 128 + 128],
                                    rhs=wdblk[:, k4, n * 384:(n + 1) * 384],
                                    start=(not d_bias and fc == 0),
                                    stop=False)
                for tt in tts:
                    nc.vector.scalar_tensor_tensor(
                        out=ffnout[:, tt, :], in0=accs[tt][:, :, 0:384],
                        scalar=1.0, in1=y[:, tt, :], op0=OP.mult, op1=OP.add,
                        accum_out=s4f[:, tt:tt + 1])
                _ln_pair(nc, pools, ffnout, s4f, tts, g2_bc, b2_bc)
                if not last:
                    _transpose_half(nc, pools, ffnout, xT_next, ident, tts)
            _act_hoist(nc, pools, AF.Exp)
            x = ffnout
            if not last:
                xT = xT_next

        for tt in range(TT):
            xo = scratch.tile([128, H], f32, tag="sc", name="out_cast")
            nc.vector.tensor_copy(xo[:], x[:, tt, :])
            nc.sync.dma_start(out_d[tt * 128:(tt + 1) * 128, :], xo[:])

    nc.compile()
    return nc


def _prep_inputs(inputs, b):
    f = np.float32
    h = np.float16
    Wq, Wk, Wv, Wo, Wi, Wd = (np.asarray(inputs[k], f)
                              for k in ("Wq", "Wk", "Wv", "Wo", "Wi", "Wd"))
    # [L, H, H] -> [l, j, k, ic, m]: out-chunk j rows=contraction chunk ic
    Wq16 = np.ascontiguousarray(
        Wq.reshape(L, HC, 128, HC, 128).transpose(0, 3, 2, 1, 4)).astype(h)
    Wk16 = np.ascontiguousarray(
        Wk.reshape(L, HC, 128, HC, 128).transpose(0, 3, 2, 1, 4)).astype(h)
    # [L, H, H] -> [l, k, ic, col]
    Wv16 = np.ascontiguousarray(
        Wv.reshape(L, HC, 128, H).transpose(0, 2, 1, 3)).astype(h)
    Wo16 = np.ascontiguousarray(
        Wo.reshape(L, HC, 128, H).transpose(0, 2, 1, 3)).astype(h)
    # [L, H, I] -> [l, p, k, q, ic, m]
    Wi16 = np.ascontiguousarray(
        Wi.reshape(L, HC, 128, FC // 2, 2, 128).transpose(0, 3, 2, 4, 1, 5)
    ).astype(h)
    # [L, I, H] -> [l, fp, k, k4, col]: quads of 128-row blocks for batched DMA
    Wd16 = np.ascontiguousarray(
        Wd.reshape(L, FC // 4, 4, 128, H).transpose(0, 1, 3, 2, 4)).astype(h)
    mask = np.asarray(inputs["input_mask"], f)
    tti = np.asarray(inputs["token_type_ids"], np.int32)
    flags = dict(
        qk_bias=bool(np.any(np.asarray(inputs["bq"])) or np.any(np.asarray(inputs["bk"]))),
        v_bias=bool(np.any(np.asarray(inputs["bv"]))),
        o_bias=bool(np.any(np.asarray(inputs["bo"]))),
        i_bias=bool(np.any(np.asarray(inputs["bi"]))),
        d_bias=bool(np.any(np.asarray(inputs["bd"]))),
        ln1_aff=bool(np.any(np.asarray(inputs["ln1_g"]) != 1.0) or
                     np.any(np.asarray(inputs["ln1_b"]))),
        ln2_aff=bool(np.any(np.asarray(inputs["ln2_g"]) != 1.0) or
                     np.any(np.asarray(inputs["ln2_b"]))),
        emb_aff=bool(np.any(np.asarray(inputs["emb_ln_g"]) != 1.0) or
                     np.any(np.asarray(inputs["emb_ln_b"]))),
        use_mask=bool(np.any(mask != 1.0)),
        use_type=bool(np.any(tti != 0)),
    )
    pos_eff = np.asarray(inputs["pos_emb"], f)[:S]
    if not flags["use_type"]:
        pos_eff = pos_eff + np.asarray(inputs["type_emb"], f)[int(tti.flat[0])][None, :]
    shared = dict(
        tok_emb=np.asarray(inputs["tok_emb"], f),
        pos_emb=pos_eff,
        type_emb=np.asarray(inputs["type_emb"], f),
        emb_g=np.asarray(inputs["emb_ln_g"], f),
        emb_b=np.asarray(inputs["emb_ln_b"], f),
        Wq16=Wq16, Wk16=Wk16, Wv16=Wv16, Wo16=Wo16, Wi16=Wi16, Wd16=Wd16,
        bq=np.asarray(inputs["bq"], f), bk=np.asarray(inputs["bk"], f),
        bv=np.asarray(inputs["bv"], f),
        bo=np.asarray(inputs["bo"], f).astype(h),
        bi=np.asarray(inputs["bi"], f),
        bd=np.asarray(inputs["bd"], f).astype(h),
        ln1_g=np.asarray(inputs["ln1_g"], f), ln1_b=np.asarray(inputs["ln1_b"], f),
        ln2_g=np.asarray(inputs["ln2_g"], f), ln2_b=np.asarray(inputs["ln2_b"], f),
        ones16=np.ones((128, 128), h),
        ident=np.eye(128, dtype=h),
    )
    in_maps = []
    ids = np.asarray(inputs["input_ids"], np.int32)
    for c in range(b):
        m = dict(shared)
        m["ids"] = np.ascontiguousarray(ids[c])
        m["tti"] = np.ascontiguousarray(tti[c])
        m["mb"] = np.ascontiguousarray((1.0 - mask[c]) * -10000.0)
        in_maps.append(m)
    return in_maps, flags


def kernel(**inputs):
    global LAST_EXEC_TIME_NS, LAST_RESULT
    n_layers = int(os.environ.get("BERT_LAYERS", L))
    trace = bool(os.environ.get("BERT_TRACE"))
    in_maps, flags = _prep_inputs(inputs, B)
    nc = build(n_layers, flags)
    res = bass_utils.run_bass_kernel_spmd(
        nc, in_maps, core_ids=list(range(B)), trace=trace)
    LAST_EXEC_TIME_NS = res.exec_time_ns
    LAST_RESULT = res
    out = np.stack([res.results[c]["out"] for c in range(B)])
    return out.astype(np.float32)
